# revision 3
# baseline (speedup 1.0000x reference)
"""Causal self-attention (single head, S=4096, D=1024) on 8 TRN2 NeuronCores.

Strategy (sequence/FlashAttention-style partitioning):
  - Queries sharded zig-zag: core c owns 256-row chunks {c, 15-c} (512 rows),
    which balances causal attention work exactly (17 key-windows of 256 per core).
  - K/V projections computed distributed (core c computes keys [512c, 512c+512))
    and AllGathered in bf16.
  - Scores computed TRANSPOSED (S^T[j, i] tiles, keys on partitions) so the
    softmax needs no max-subtraction (scores are O(5)), probabilities feed the
    PV matmul directly as lhsT (no on-chip transposes), and row sums come from
    a ones-vector matmul.
  - Per-core worklists (key-window offsets, query-chunk selects, mask selects)
    are int32 tables driving register-based dynamic addressing, so all 8 cores
    run one identical SPMD program.
"""

import numpy as np
import ml_dtypes

S = 4096
D = 1024
N_CORES = 8
P = 128
CHUNK = 256           # query chunk rows (one i-block)
WIN = 256             # key window
N_SLOTS = 17          # uniform per-core key-window count
KT_ELEMS = D * 512          # per-rank kT block elements in AG buffer
V_ELEMS = 512 * D           # per-rank v block elements
RANK_ELEMS = KT_ELEMS + V_ELEMS
AG_OUT_ELEMS = N_CORES * RANK_ELEMS
SCALE = 1.0 / 32.0          # 1/sqrt(D)

_CACHE = {}


def _build():
    import concourse.bass as bass
    import concourse.mybir as mybir
    import concourse.tile as tile
    from concourse import bacc
    from concourse.bass import ds

    bf16 = mybir.dt.bfloat16
    f32 = mybir.dt.float32
    i32 = mybir.dt.int32

    nc = bacc.Bacc("TRN2", target_bir_lowering=False, debug=False,
                   num_devices=N_CORES)

    # ---- per-core I/O ----
    wq = nc.dram_tensor("wq", [P, 8, D], bf16, kind="ExternalInput")
    wk = nc.dram_tensor("wk", [P, 8, D], bf16, kind="ExternalInput")
    wv = nc.dram_tensor("wv", [P, 8, D], bf16, kind="ExternalInput")
    xq = nc.dram_tensor("xq", [P, 8, 512], bf16, kind="ExternalInput")   # x^T, own q rows
    xkv = nc.dram_tensor("xkv", [P, 8, 512], bf16, kind="ExternalInput")  # x^T, own kv rows
    tabd = nc.dram_tensor("tab", [1, N_SLOTS * 8], i32, kind="ExternalInput")
    maskd = nc.dram_tensor("mask", [P, 512], bf16, kind="ExternalInput")
    onesd = nc.dram_tensor("ones", [P, 1], bf16, kind="ExternalInput")
    outd = nc.dram_tensor("out", [512, D], f32, kind="ExternalOutput")

    agin = nc.dram_tensor("agin", [1, RANK_ELEMS], bf16)
    agout = nc.dram_tensor("agout", [1, AG_OUT_ELEMS], bf16, addr_space="Shared")
    rs_dram = nc.dram_tensor("rs_dram", [1, 512], f32)

    def dyn_view(flat_dram, off_scalar, pattern):
        a = flat_dram[0:1, ds(off_scalar, 1)]
        return bass.AP(a.tensor, a.offset, pattern)

    with tile.TileContext(nc) as tc:
        with tc.tile_pool(name="wpool", bufs=3) as wpool, \
             tc.tile_pool(name="xpool", bufs=2) as xpool, \
             tc.tile_pool(name="qt", bufs=1) as qtpool, \
             tc.tile_pool(name="stage", bufs=3) as stage, \
             tc.tile_pool(name="consts", bufs=1) as consts, \
             tc.tile_pool(name="accs", bufs=1) as accs:

            # ---------------- Phase 1: projections ----------------
            xkv_sb = xpool.tile([P, 8, 512], bf16, name="xkv_sb")
            nc.sync.dma_start(xkv_sb[:], xkv[:])
            wk_sb = wpool.tile([P, 8, D], bf16, name="wk_sb", tag="w")
            nc.sync.dma_start(wk_sb[:], wk[:])
            wv_sb = wpool.tile([P, 8, D], bf16, name="wv_sb", tag="w")
            nc.sync.dma_start(wv_sb[:], wv[:])
            wq_sb = wpool.tile([P, 8, D], bf16, name="wq_sb", tag="w")
            nc.sync.dma_start(wq_sb[:], wq[:])
            xq_sb = xpool.tile([P, 8, 512], bf16, name="xq_sb")
            nc.sync.dma_start(xq_sb[:], xq[:])

            with tc.tile_pool(name="pps", bufs=2, space="PSUM") as pps, \
                 tc.tile_pool(name="ppsv", bufs=2, space="PSUM") as ppsv:
                # kT_c: [8 dko][128 dp][512 j] into agin[0 : KT_ELEMS]
                for dt in range(8):
                    ps = pps.tile([P, 512], f32, name=f"kt_ps{dt}", tag="ktps")
                    for ko in range(8):
                        nc.tensor.matmul(ps[:], wk_sb[:, ko, dt * P:(dt + 1) * P],
                                         xkv_sb[:, ko, :],
                                         start=(ko == 0), stop=(ko == 7))
                    st = stage.tile([P, 512], bf16, name=f"kt_st{dt}", tag="ktst")
                    nc.vector.tensor_copy(st[:], ps[:])
                    dst = bass.AP(agin, dt * (P * 512),
                                  [[512, P], [1, 512]])
                    nc.sync.dma_start(dst, st[:])

                # v_c: [512 s][1024 d] into agin[KT_ELEMS : ]
                for st_i in range(4):
                    ps = ppsv.tile([P, D], f32, name=f"v_ps{st_i}", tag="vps")
                    for ko in range(8):
                        for dh in range(2):
                            nc.tensor.matmul(
                                ps[:, dh * 512:(dh + 1) * 512],
                                xkv_sb[:, ko, st_i * P:(st_i + 1) * P],
                                wv_sb[:, ko, dh * 512:(dh + 1) * 512],
                                start=(ko == 0), stop=(ko == 7))
                    st = stage.tile([P, D], bf16, name=f"v_st{st_i}", tag="vst")
                    nc.vector.tensor_copy(st[:], ps[:])
                    dst = bass.AP(agin, KT_ELEMS + st_i * (P * D),
                                  [[D, P], [1, D]])
                    nc.sync.dma_start(dst, st[:])

                # ---------------- Phase 2: AllGather K/V ----------------
                nc.gpsimd.collective_compute(
                    "AllGather", mybir.AluOpType.bypass,
                    replica_groups=[list(range(N_CORES))],
                    ins=[agin.ap().opt()],
                    outs=[agout.ap().opt()],
                )

                # qT_c: keep in SBUF [128 dp, 8 dko, 512 i]
                qt_sb = qtpool.tile([P, 8, 512], bf16, name="qt_sb")
                for dt in range(8):
                    ps = pps.tile([P, 512], f32, name=f"q_ps{dt}", tag="ktps")
                    for ko in range(8):
                        nc.tensor.matmul(ps[:], wq_sb[:, ko, dt * P:(dt + 1) * P],
                                         xq_sb[:, ko, :],
                                         start=(ko == 0), stop=(ko == 7))
                    nc.vector.tensor_copy(qt_sb[:, dt, :], ps[:])

            # ---------------- Phase 3: attention ----------------
            tab_sb = consts.tile([1, N_SLOTS * 8], i32, name="tab_sb")
            nc.sync.dma_start(tab_sb[:], tabd[:])
            mask_sb = consts.tile([P, 512], bf16, name="mask_sb")
            nc.sync.dma_start(mask_sb[:], maskd[:])
            ones_sb = consts.tile([P, 1], bf16, name="ones_sb")
            nc.sync.dma_start(ones_sb[:], onesd[:])

            acc_out = accs.tile([P, 4 * D], f32, name="acc_out")
            nc.vector.memset(acc_out[:], 0.0)
            acc_rs = accs.tile([1, 512], f32, name="acc_rs")
            nc.vector.memset(acc_rs[:], 0.0)

            ALL = mybir.ALL_ENGINES
            PE = [mybir.EngineType.PE]
            DVE = [mybir.EngineType.DVE]

            with tc.tile_pool(name="ktw", bufs=2) as ktw, \
                 tc.tile_pool(name="vw", bufs=2) as vw, \
                 tc.tile_pool(name="pt", bufs=3) as ptp, \
                 tc.tile_pool(name="stps", bufs=2, space="PSUM") as stps, \
                 tc.tile_pool(name="pvps", bufs=1, space="PSUM") as pvps, \
                 tc.tile_pool(name="rsps", bufs=2, space="PSUM") as rsps:

                for t in range(N_SLOTS):
                    def tv(k, mx, eng=ALL):
                        return nc.values_load(
                            tab_sb[0:1, t * 8 + k: t * 8 + k + 1],
                            engines=eng, min_val=0, max_val=mx,
                            skip_runtime_bounds_check=True)

                    kt_off = tv(0, AG_OUT_ELEMS - KT_ELEMS + 256)
                    v_off = tv(1, AG_OUT_ELEMS - 256 * D)
                    qoff = tv(2, 256, PE)
                    m0 = tv(3, 256, DVE)
                    m1 = tv(4, 256, DVE)
                    a0 = tv(5, 3 * D, DVE)
                    a1 = tv(6, 3 * D, DVE)
                    rs_off = tv(7, 256, DVE)

                    kt_w = ktw.tile([P, 8, WIN], bf16, name=f"kt_w{t}", tag="ktw")
                    nc.sync.dma_start(
                        kt_w[:], dyn_view(agin if False else agout, kt_off,
                                          [[512, P], [KT_ELEMS // 8, 8], [1, WIN]]))
                    v_w = vw.tile([P, 2, D], bf16, name=f"v_w{t}", tag="vw")
                    nc.sync.dma_start(
                        v_w[:], dyn_view(agout, v_off,
                                         [[D, P], [P * D, 2], [1, D]]))

                    pv_ps = pvps.tile([P, 2, D], f32, name=f"pv{t}", tag="pv")
                    rs_ps = rsps.tile([1, 256], f32, name=f"rs{t}", tag="rs")
                    pts = []
                    for js in range(2):
                        st_ps = stps.tile([P, 256], f32, name=f"st{t}_{js}", tag="st")
                        for ko in range(8):
                            nc.tensor.matmul(
                                st_ps[:],
                                kt_w[:, ko, js * P:(js + 1) * P],
                                qt_sb[:, ko, ds(qoff, 256)],
                                start=(ko == 0), stop=(ko == 7))
                        pt = ptp.tile([P, 256], bf16, name=f"pt{t}_{js}", tag="pt")
                        nc.scalar.activation(pt[:], st_ps[:],
                                             mybir.ActivationFunctionType.Exp,
                                             scale=SCALE)
                        moff = m0 if js == 0 else m1
                        nc.vector.tensor_mul(pt[:], pt[:], mask_sb[:, ds(moff, 256)])
                        pts.append(pt)

                        nc.tensor.matmul(rs_ps[:], ones_sb[:], pt[:],
                                         start=(js == 0), stop=(js == 1))
                        for isub in range(2):
                            for dh in range(2):
                                nc.tensor.matmul(
                                    pv_ps[:, isub, dh * 512:(dh + 1) * 512],
                                    pt[:, isub * P:(isub + 1) * P],
                                    v_w[:, js, dh * 512:(dh + 1) * 512],
                                    start=(js == 0), stop=(js == 1))

                    for isub, a_off in ((0, a0), (1, a1)):
                        nc.vector.tensor_add(acc_out[:, ds(a_off, D)],
                                             acc_out[:, ds(a_off, D)],
                                             pv_ps[:, isub, :])
                    nc.vector.tensor_add(acc_rs[0:1, ds(rs_off, 256)],
                                         acc_rs[0:1, ds(rs_off, 256)],
                                         rs_ps[:])

                # ---------------- finalize: divide by row sums ----------------
                # bounce [1, 512] through DRAM to scatter i = ib*128+p onto
                # partitions as [128 p, 4 ib]
                nc.vector.reciprocal(acc_rs[:], acc_rs[:])
                nc.sync.dma_start(rs_dram.ap(), acc_rs[:])
                recipT = consts.tile([P, 4], f32, name="recipT")
                nc.sync.dma_start(
                    recipT[:],
                    rs_dram.ap().rearrange("o (ib p) -> (o p) ib", p=P))
                o_f32 = accs.tile([P, 4, D], f32, name="o_f32")
                av = acc_out[:].rearrange("p (ib d) -> p ib d", ib=4)
                for ib in range(4):
                    nc.vector.tensor_tensor(
                        o_f32[:, ib, :], av[:, ib, :],
                        recipT[:, ib, None].to_broadcast((P, 1, D)),
                        mybir.AluOpType.mult)
                nc.sync.dma_start(
                    outd.ap().rearrange("(ib p) d -> p ib d", p=P), o_f32[:])

    nc.compile()
    return nc


def _host_inputs(x, W_query, W_key, W_value):
    """Builds the 8 per-core input maps."""
    bf = ml_dtypes.bfloat16

    def wprep(W):
        # [1024 k, 1024 d] -> [128 kp, 8 ko, 1024 d]
        return np.ascontiguousarray(
            W.reshape(8, P, D).transpose(1, 0, 2)).astype(bf)

    wq_n, wk_n, wv_n = wprep(W_query), wprep(W_key), wprep(W_value)

    def xprep(rows):
        # x[rows] [512, 1024] -> x^T [128 kp, 8 ko, 512 s]
        xt = x[rows].T.reshape(8, P, 512).transpose(1, 0, 2)
        return np.ascontiguousarray(xt).astype(bf)

    # mask atlas [128, 512]: [ZERO | TRI | ONES | ONES]
    mask = np.zeros((P, 512), dtype=bf)
    tri = (np.arange(P)[:, None] <= np.arange(P)[None, :])
    mask[:, 128:256] = tri.astype(bf)
    mask[:, 256:512] = np.ones((P, 256), dtype=bf)
    ones = np.ones((P, 1), dtype=bf)

    in_maps = []
    for c in range(N_CORES):
        A, B = c, 15 - c
        q_rows = np.r_[CHUNK * A:CHUNK * A + CHUNK, CHUNK * B:CHUNK * B + CHUNK]
        kv_rows = np.arange(512 * c, 512 * c + 512)
        slots = [(0, w) for w in range(A + 1)] + [(1, w) for w in range(B + 1)]
        assert len(slots) == N_SLOTS
        tab = np.zeros((1, N_SLOTS * 8), dtype=np.int32)
        for t, (ch, w) in enumerate(slots):
            i0 = CHUNK * (A if ch == 0 else B)
            diag = (WIN * w == i0)
            rank, jloc = w // 2, WIN * (w % 2)
            tab[0, t * 8 + 0] = rank * RANK_ELEMS + jloc
            tab[0, t * 8 + 1] = rank * RANK_ELEMS + KT_ELEMS + jloc * D
            qoff = 256 * ch
            tab[0, t * 8 + 2] = qoff
            tab[0, t * 8 + 3] = 128 if diag else 256
            tab[0, t * 8 + 4] = 0 if diag else 256
            tab[0, t * 8 + 5] = qoff * 8
            tab[0, t * 8 + 6] = qoff * 8 + D
            tab[0, t * 8 + 7] = qoff
        in_maps.append({
            "wq": wq_n, "wk": wk_n, "wv": wv_n,
            "xq": xprep(q_rows), "xkv": xprep(kv_rows),
            "tab": tab, "mask": mask, "ones": ones,
        })
    return in_maps


def kernel(x, W_query, W_key, W_value):
    from concourse.bass_utils import run_bass_kernel_spmd

    x = np.asarray(x, dtype=np.float32)
    W_query = np.asarray(W_query, dtype=np.float32)
    W_key = np.asarray(W_key, dtype=np.float32)
    W_value = np.asarray(W_value, dtype=np.float32)

    if "nc" not in _CACHE:
        _CACHE["nc"] = _build()
    nc = _CACHE["nc"]

    in_maps = _host_inputs(x, W_query, W_key, W_value)
    res = run_bass_kernel_spmd(nc, in_maps, core_ids=list(range(N_CORES)))

    out = np.empty((S, D), dtype=np.float32)
    for c in range(N_CORES):
        o = res.results[c]["out"]
        A, B = c, 15 - c
        out[CHUNK * A:CHUNK * A + CHUNK] = o[0:CHUNK]
        out[CHUNK * B:CHUNK * B + CHUNK] = o[CHUNK:512]
    return out
